# revision 20
# baseline (speedup 1.0000x reference)
"""DCNv2 on 8 trn2 cores, data-parallel over batch. v3: batched projection
copies + all-scalar PSUM drains + optional dynamic outer-term skipping.

Per core (one image):
  phase1: offset/mask convs (z-chunk + x-chunk PSUM accumulation), OM bf16
  phase2: PE-transpose OM -> OMT[j, 27, i] bf16 (copies batched 4 rows)
  phase2.5: outer-term flags for ALL bands at once:
    |d|>0.99 -> relu -> ones-matmul -> reduces -> FLS[1, 2, 3, 16] (ax, kx, band)
  per band (R=8 rows):
    projection: per source row ONE [128,1728] f32 PSUM tile (7 bank-safe
      matmuls across the 5 sx shifts) + ONE scalar copy into VT2
      [j, pair, ky, o, s(16)]; outer rows ONE [128,1536] tile (6 pair-
      padded matmuls) + ONE scalar copy into VT3 (slot-contiguous so the
      MAC runs i-inner at DVE 2x)
    tents: TY/TX (scalar) -> TYM=TY*MSK (DVE) -> CT2=TYM*TX (DVE)
      (the deform-conv mask factor 2 is folded into wflat on the host)
    MAC: all-DVE bf16 tensor_tensor mult+add pairs, dims (ky,o,i), one ACC
      inner 27 terms unconditional; 36 outer (|e|=2) terms per kx gated on
      FLS via tc.If when DYN_SKIP (tents are exactly 0 when no |d|>1)
    collapse ky-sum, DMA out
"""
import sys

sys.path.insert(0, "/opt/trn_rl_repo")

import numpy as np

import concourse.bass as bass
import concourse.mybir as mybir
import concourse.tile as tile
from concourse.bass_utils import run_bass_kernel_spmd

F32 = mybir.dt.float32
BF16 = mybir.dt.bfloat16
ALU = mybir.AluOpType
ACTF = mybir.ActivationFunctionType

H = W = 128
C = O = 64
KK = 9
PW = 134          # padded width/height, image at [3, 131)
R = 8             # output rows per band
NBANDS = H // R
SLOTS = 14        # source rows per band: padded rows [i0, i0+14)
SLEN = 15         # slot-dim storage (slots 1..14 used)
S0 = 1
GB = 4            # bands per tent-group
NCORES = 8
DYN_SKIP = True

# (sx, kx) pairs, sx = kx - 1 + ex.  main: |ex|<=1, outer: |ex|=2
MAIN_SX = {}      # sx -> (kxlo, kxhi)
for sx in range(-2, 3):
    kxs = [kx for kx in range(3) if abs(sx - kx + 1) <= 1]
    MAIN_SX[sx] = (min(kxs), max(kxs))
MAIN_PAIRS = []   # ordered (sx, kx)
for sx in range(-2, 3):
    lo, hi = MAIN_SX[sx]
    for kx in range(lo, hi + 1):
        MAIN_PAIRS.append((sx, kx))
MAIN_IDX = {p: i for i, p in enumerate(MAIN_PAIRS)}        # 9 pairs
OUT_PAIRS = [(kx - 1 + ex, kx) for kx in range(3) for ex in (-2, 2)]
OUT_IDX = {p: i for i, p in enumerate(OUT_PAIRS)}          # 6 pairs
PBLK = 3 * O * SLEN   # 3072 elems per (sx,kx) pair block
KYSTR = O * SLEN      # 1024
SLEN3 = 12            # VT3 slot dim: stores s-2 for s in [2,14)
PBLK3 = 3 * O * SLEN3
KYSTR3 = O * SLEN3

INNER = [(ey, ex) for ey in (-1, 0, 1) for ex in (-1, 0, 1)]
OUTER_Y = [(ey, ex) for ey in (-2, 2) for ex in (-1, 0, 1)]
OUTER_X = [(ey, ex) for ey in (-1, 0, 1) for ex in (-2, 2)]

# Projections go through [128, 2, half-row] f32 PSUM tiles (two source rows
# per tile) so ONE scalar copy per half drains both rows with innermost
# [1,2] pair-writes into VT2/VT3 (measured ~0.97ns/elem vs 4.4 for single-
# slot scatter).  Matmul chunks must stay inside one 2KB PSUM bank (512
# f32) and inside one sx block.


def _bank_chunks(blocks, base):
    """blocks: (lo, hi, sx, wf0) in local cols; emit (c0, c1, sx, wfc) in
    tile cols (local + base), split at 512-f32 bank boundaries."""
    out = []
    for lo, hi, sx, wf0 in blocks:
        c = base + lo
        end = base + hi
        while c < end:
            nb = min(end, (c // 512 + 1) * 512)
            out.append((c, nb, sx, wf0 + (c - (base + lo))))
            c = nb
    return out


# main half A: pairs 0-4 (cols 0..960 of the 1728), half B: pairs 5-8
MAIN_A_BLOCKS = [(0, 192, -2, 0), (192, 576, -1, 0), (576, 960, 0, 0)]
MAIN_B_BLOCKS = [(0, 192, 0, 384), (192, 576, 1, 192), (576, 768, 2, 384)]
MAIN_A_CH = [_bank_chunks(MAIN_A_BLOCKS, 0), _bank_chunks(MAIN_A_BLOCKS, 960)]
MAIN_B_CH = [_bank_chunks(MAIN_B_BLOCKS, 0), _bank_chunks(MAIN_B_BLOCKS, 768)]
# outer halves: OUT_PAIRS[0:3] and [3:6], 192 cols each, unpadded
def _out_ch(pairs, base):
    return _bank_chunks(
        [(i * 192, i * 192 + 192, sx, kx * 192) for i, (sx, kx) in
         enumerate(pairs)], base)
OUT_A_CH = [_out_ch(OUT_PAIRS[0:3], 0), _out_ch(OUT_PAIRS[0:3], 576)]
OUT_B_CH = [_out_ch(OUT_PAIRS[3:6], 0), _out_ch(OUT_PAIRS[3:6], 576)]


def _fix_multiwait(nc, max_waits=1):
    import bass_rust

    ctr = 0
    for f in nc.m.functions:
        for bb in f.blocks:
            insts = bb.instructions

            def nwaits(i):
                si = i.sync_info
                return len(si.on_wait) if si is not None else 0

            if not any(nwaits(i) > max_waits for i in insts):
                continue
            out = []
            for inst in insts:
                si = inst.sync_info
                waits = list(si.on_wait) if si is not None else []
                if len(waits) > max_waits:
                    extra, keep = waits[:-max_waits], waits[-max_waits:]
                    for j in range(0, len(extra), max_waits):
                        ctr += 1
                        nop = mybir.InstNoOp(name=f"WFIX-{ctr}", ins=[], outs=[])
                        nop.engine = inst.engine
                        nop.sync_info = bass_rust.SyncInfo(
                            on_wait=extra[j : j + max_waits], on_update=[]
                        )
                        out.append(nop)
                    inst.sync_info = bass_rust.SyncInfo(
                        on_wait=keep, on_update=list(si.on_update)
                    )
                out.append(inst)
            bb.instructions = out


def build_nc(fix_waits=True, dyn_skip=DYN_SKIP):
    nc = bass.Bass()
    zin = nc.dram_tensor("zin", [64, PW * PW], BF16, kind="ExternalInput")
    xin = nc.dram_tensor("xin", [64, PW * PW], BF16, kind="ExternalInput")
    wcz = nc.dram_tensor("wcz", [64, KK * 27], BF16, kind="ExternalInput")
    wcx = nc.dram_tensor("wcx", [64, KK * 27], BF16, kind="ExternalInput")
    wflat = nc.dram_tensor("wflat", [C, 3 * 3 * O], BF16, kind="ExternalInput")
    bias27 = nc.dram_tensor("bias27", [27, 1], F32, kind="ExternalInput")
    identb = nc.dram_tensor("identb", [32, 32], BF16, kind="ExternalInput")
    cstb = nc.dram_tensor("cstb", [128, 8], BF16, kind="ExternalInput")
    onesb = nc.dram_tensor("onesb", [128, 1], BF16, kind="ExternalInput")
    outD = nc.dram_tensor("outD", [128, NBANDS, O * R], BF16, kind="ExternalOutput")

    def rap(t, off, dims):
        a = t[:]
        return bass.AP(tensor=a.tensor, offset=a.offset + off, ap=dims)

    with tile.TileContext(nc) as tc:
        with tc.tile_pool(name="persist", bufs=1) as pp:
            X = pp.tile([64, PW, PW], BF16)
            WF = pp.tile([64, 3 * 3 * O], BF16)   # [c, kx*192 + ky*64 + o]
            OMT = pp.tile([128, 27, H], BF16)     # [j, plane, i]
            CSTB = pp.tile([128, 8], BF16)        # [-2,-1,0,1,2, 1.0, -0.99, 0]
            ONES = pp.tile([128, 1], BF16)
            BIA = pp.tile([27, 1], F32)
            IDTB = pp.tile([32, 32], BF16)
            FLS = pp.tile([1, 2, 3, NBANDS], F32)  # (ax, kx, band) flag counts
            MSKA = pp.tile([128, 9, H], BF16)      # 2x-folded mask, all bands
            nc.sync.dma_start(X[:], xin.rearrange("p (a b) -> p a b", b=PW))
            nc.sync.dma_start(WF[:], wflat[:])
            nc.sync.dma_start(CSTB[:], cstb[:])
            nc.sync.dma_start(ONES[:], onesb[:])
            nc.sync.dma_start(BIA[:], bias27[:])
            nc.sync.dma_start(IDTB[:], identb[:])

            # ---- phase 1+2: offset/mask convs, then transpose to OMT
            with (
                tc.tile_pool(name="ph1", bufs=1) as p1,
                tc.tile_pool(name="psc", bufs=2, space="PSUM") as pconv,
                tc.tile_pool(name="pst", bufs=2, space="PSUM") as ptr,
            ):
                # z on partitions 0-63, x on 64-127: one 128-deep matmul
                # per tap does both convs (halves matmul+LDW count)
                ZX = p1.tile([128, PW, PW], BF16)
                WCZX = p1.tile([128, KK, 27], BF16)
                OM = p1.tile([27, H, W], BF16)
                nc.sync.dma_start(ZX[0:64], zin.rearrange("p (a b) -> p a b", b=PW))
                nc.sync.dma_start(ZX[64:128], xin.rearrange("p (a b) -> p a b", b=PW))
                nc.sync.dma_start(WCZX[0:64], wcz.rearrange("p (t q) -> p t q", q=27))
                nc.sync.dma_start(WCZX[64:128], wcx.rearrange("p (t q) -> p t q", q=27))
                for nt in range(32):  # 4 image rows per PSUM tile
                    r0 = nt * 4
                    ps = pconv.tile([27, 512], F32, tag="convps")
                    for t in range(KK):
                        ty, tx = t // 3, t % 3
                        zxr = ZX[:, r0 + 2 + ty : r0 + 6 + ty, 2 + tx : 2 + tx + W]
                        nc.tensor.matmul(
                            ps[:], WCZX[:, t, :], zxr,
                            start=(t == 0), stop=(t == KK - 1),
                        )
                    ps3 = ps[:].rearrange("p (a b) -> p a b", b=W)
                    nc.scalar.activation(
                        OM[:, r0 : r0 + 4, :], ps3, ACTF.Identity, bias=BIA[:, 0:1]
                    )
                for i4 in range(H // 4):
                    pt = ptr.tile([128, 4, 32], BF16, tag="trps")
                    for k in range(4):
                        nc.tensor.transpose(
                            pt[:, k, 0:27], OM[:, i4 * 4 + k, :], IDTB[0:27, 0:27]
                        )
                    nc.scalar.copy(
                        OMT[:, :, i4 * 4 : i4 * 4 + 4],
                        rap(pt, 0, [[128, 128], [1, 27], [32, 4]]),
                    )

            # ---- phase 2.5: outer-term flags for all bands
            with (
                tc.tile_pool(name="flg", bufs=1) as pf,
                tc.tile_pool(name="psfl", bufs=1, space="PSUM") as pfp,
            ):
                AB = pf.tile([128, 18, H], BF16)
                nc.scalar.activation(AB[:], OMT[:, 0:18, :], ACTF.Abs)
                MV = pf.tile([128, 18, H], BF16)
                nc.scalar.activation(MV[:], AB[:], ACTF.Relu, bias=CSTB[:, 6:7])
                psF = pfp.tile([1, 18 * H], F32)
                for c0 in range(0, 18 * H, 512):
                    c1 = min(c0 + 512, 18 * H)
                    nc.tensor.matmul(
                        psF[:, c0:c1], ONES[:, 0:1],
                        rap(MV, c0, [[18 * H, 128], [1, c1 - c0]]),
                        start=True, stop=True,
                    )
                T1 = pf.tile([1, 18, NBANDS], F32)
                nc.vector.tensor_reduce(
                    out=T1[:],
                    in_=psF[:].rearrange("p (a b c) -> p a b c", b=NBANDS, c=R),
                    axis=mybir.AxisListType.X, op=ALU.add,
                )
                for ax in range(2):  # sum the 3 ky taps per kx group
                    nc.vector.tensor_reduce(
                        out=FLS[0:1, ax],
                        in_=rap(T1, ax * 9 * NBANDS,
                                [[18 * NBANDS, 1], [NBANDS, 3], [1, NBANDS],
                                 [3 * NBANDS, 3]]),
                        axis=mybir.AxisListType.X, op=ALU.add,
                    )
                # mask for all bands at once (keeps Sigmoid table loads out
                # of the band loop)
                nc.scalar.activation(MSKA[:], OMT[:, 18:27, :], ACTF.Sigmoid)

            # ---- phase 3: per-band
            with (
                tc.tile_pool(name="vt2p", bufs=2) as pvt2,
                tc.tile_pool(name="vt3p", bufs=1) as pvt3,
                tc.tile_pool(name="band", bufs=2) as pb,
                tc.tile_pool(name="bscr", bufs=1) as pb1,
                tc.tile_pool(name="accs", bufs=1) as pacc,
                tc.tile_pool(name="psv", bufs=2, space="PSUM") as pproj,
            ):
                for ib in range(NBANDS):
                    i0 = ib * R

                    # -- tents, computed once per GB-band group
                    GBR = GB * R
                    if ib % GB == 0:
                        g0 = i0
                        # TYX holds tent_y (taps 0-8) and tent_x (taps 9-17)
                        TYX = pb.tile([128, 18, 5, GBR], BF16, tag="tyx")
                        TA4 = pb1.tile([128, 18, GBR], BF16, tag="ta4")
                        one = CSTB[:, 5:6]
                        for e in range(5):
                            nege = CSTB[:, 4 - e : 5 - e]
                            nc.scalar.activation(
                                TA4[:], OMT[:, 0:18, g0 : g0 + GBR], ACTF.Abs,
                                bias=nege,
                            )
                            nc.scalar.activation(
                                TYX[:, :, e, :], TA4[:], ACTF.Relu, bias=one,
                                scale=-1.0,
                            )
                    go = (ib % GB) * R
                    TYM = pb.tile([128, 9, 5, R], BF16, tag="tym")
                    nc.gpsimd.tensor_tensor(
                        out=TYM[:],
                        in0=rap(TYX, go, [[18 * 5 * GBR, 128], [5 * GBR, 9],
                                          [GBR, 5], [1, R]]),
                        in1=rap(MSKA, i0, [[9 * H, 128], [H, 9], [0, 5],
                                           [1, R]]),
                        op=ALU.mult,
                    )
                    # CT2[j, kx, ey, ex, ky, i] = TYM * TX  (x2 folded in wflat)
                    CT2 = pb.tile([128, 3, 5, 5, 3, R], BF16, tag="ct2")
                    for kx in range(3):
                        for eyi in range(5):
                            nc.gpsimd.tensor_tensor(
                                out=CT2[:, kx, eyi],
                                in0=rap(TYM, kx * 5 * R + eyi * R,
                                        [[9 * 5 * R, 128], [0, 5], [15 * R, 3],
                                         [1, R]]),
                                in1=rap(TYX, (9 + kx) * 5 * GBR + go,
                                        [[18 * 5 * GBR, 128], [GBR, 5],
                                         [15 * GBR, 3], [1, R]]),
                                op=ALU.mult,
                            )

                    # -- projections: two source rows per PSUM tile, one
                    # scalar pair-write copy per half-tile (innermost [1,2]
                    # dest runs measure ~0.97ns/elem vs 4.4 for single-slot
                    # scatter).  Main rows first, then outer (VT3 copies wait
                    # for the previous band's DUM touch, early under dyn_skip).
                    VT2 = pvt2.tile([128, 9 * PBLK], BF16, tag="vt2")
                    VT3 = pvt3.tile([128, 6 * PBLK3], BF16, tag="vt3")

                    def mm2(ps, npair, chunks_per_row, rows):
                        flat = rap(ps, 0, [[2 * npair * 192, 128],
                                           [1, 2 * npair * 192]])
                        for k, prow in enumerate(rows):
                            for c0, c1, sx, wfc in chunks_per_row[k]:
                                nc.tensor.matmul(
                                    bass.AP(tensor=flat.tensor,
                                            offset=flat.offset + c0,
                                            ap=[[2 * npair * 192, 128],
                                                [1, c1 - c0]]),
                                    X[:, prow, 3 + sx : 3 + sx + W],
                                    WF[:, wfc : wfc + (c1 - c0)],
                                    start=True, stop=True,
                                )

                    for g in range(SLOTS // 2):
                        s = S0 + 2 * g
                        rows = (i0 + 2 * g, i0 + 2 * g + 1)
                        psA = pproj.tile([128, 2, 960], F32, tag="vtps")
                        mm2(psA, 5, MAIN_A_CH, rows)
                        nc.scalar.copy(
                            rap(VT2, s,
                                [[9 * PBLK, 128], [KYSTR, 15], [SLEN, O],
                                 [1, 2]]),
                            rap(psA, 0, [[1920, 128], [O, 15], [1, O],
                                         [960, 2]]),
                        )
                        psB = pproj.tile([128, 2, 768], F32, tag="vtps")
                        mm2(psB, 4, MAIN_B_CH, rows)
                        nc.scalar.copy(
                            rap(VT2, s + 15 * KYSTR,
                                [[9 * PBLK, 128], [KYSTR, 12], [SLEN, O],
                                 [1, 2]]),
                            rap(psB, 0, [[1536, 128], [O, 12], [1, O],
                                         [768, 2]]),
                        )
                    # outer pairs (VT3 only serves s in [2,14))
                    for g in range(6):
                        s = 2 + 2 * g
                        rows = (i0 + s - S0, i0 + s - S0 + 1)
                        psOA = pproj.tile([128, 2, 576], F32, tag="vtps")
                        mm2(psOA, 3, OUT_A_CH, rows)
                        nc.scalar.copy(
                            rap(VT3, s - 2,
                                [[6 * PBLK3, 128], [KYSTR3, 9], [SLEN3, O],
                                 [1, 2]]),
                            rap(psOA, 0, [[1152, 128], [O, 9], [1, O],
                                          [576, 2]]),
                        )
                        psOB = pproj.tile([128, 2, 576], F32, tag="vtps")
                        mm2(psOB, 3, OUT_B_CH, rows)
                        nc.scalar.copy(
                            rap(VT3, (s - 2) + 9 * KYSTR3,
                                [[6 * PBLK3, 128], [KYSTR3, 9], [SLEN3, O],
                                 [1, 2]]),
                            rap(psOB, 0, [[1152, 128], [O, 9], [1, O],
                                          [576, 2]]),
                        )

                    # -- MAC (all DVE).  Outer terms run FIRST into a zeroed
                    # ACCB so the VT3 WAR (bufs=1) clears early and the next
                    # band's scalar copies overlap this band's inner MAC.
                    ACCA = pacc.tile([128, 3, O, R], BF16, tag="acca")
                    TMPA = pacc.tile([128, 3, O, R], BF16, tag="tmpa")
                    ACCB = pb.tile([128, 3, O, R], BF16, tag="accb")

                    def vap(kx, ey, ex):
                        sx = kx - 1 + ex
                        if abs(ex) <= 1:
                            return rap(
                                VT2, MAIN_IDX[(sx, kx)] * PBLK + (ey + 2 + S0),
                                [[9 * PBLK, 128], [KYSTR + 1, 3], [SLEN, O],
                                 [1, R]],
                            )
                        return rap(
                            VT3, OUT_IDX[(sx, kx)] * PBLK3 + (ey + 1),
                            [[6 * PBLK3, 128], [KYSTR3 + 1, 3], [SLEN3, O],
                             [1, R]],
                        )

                    def cap(kx, ey, ex):
                        off = ((kx * 5 + (ey + 2)) * 5 + (ex + 2)) * 3 * R
                        return rap(
                            CT2, off,
                            [[3 * 5 * 5 * 3 * R, 128], [R, 3], [0, O], [1, R]],
                        )

                    def term(eng, acc, tmp, kx, ey, ex, first):
                        if first:
                            eng.tensor_tensor(
                                out=acc[:], in0=vap(kx, ey, ex),
                                in1=cap(kx, ey, ex), op=ALU.mult,
                            )
                        else:
                            eng.tensor_tensor(
                                out=tmp[:], in0=vap(kx, ey, ex),
                                in1=cap(kx, ey, ex), op=ALU.mult,
                            )
                            eng.tensor_tensor(
                                out=acc[:], in0=acc[:], in1=tmp[:], op=ALU.add
                            )

                    # outer terms first, dynamically skipped per (axis, kx)
                    # (memset on the otherwise-idle gpsimd engine)
                    nc.gpsimd.memset(ACCB[:], 0.0)
                    if dyn_skip:
                        nc.vector.drain()
                        for kx in range(3):
                            rf = nc.vector.alloc_register(f"fy{ib}_{kx}")
                            nc.vector.reg_load(
                                rf, FLS[0:1, 0, kx, ib : ib + 1].bitcast(
                                    mybir.dt.int32)
                            )
                            with tc.If(bass.RuntimeValue(rf) == 0) as cy:
                                pass
                            with cy.Else():
                                for ey, ex in OUTER_Y:
                                    term(nc.vector, ACCB, TMPA, kx, ey, ex, False)
                            nc.vector.free_register(rf)
                            rf = nc.vector.alloc_register(f"fx{ib}_{kx}")
                            nc.vector.reg_load(
                                rf, FLS[0:1, 1, kx, ib : ib + 1].bitcast(
                                    mybir.dt.int32)
                            )
                            with tc.If(bass.RuntimeValue(rf) == 0) as cx:
                                pass
                            with cx.Else():
                                for ey, ex in OUTER_X:
                                    term(nc.vector, ACCB, TMPA, kx, ey, ex, False)
                            nc.vector.free_register(rf)
                    else:
                        for kx in range(3):
                            for ey, ex in OUTER_Y + OUTER_X:
                                term(nc.vector, ACCB, TMPA, kx, ey, ex, False)
                    # inner terms (always); the unconditional VT3 touch (WAR
                    # anchor for next band's writes) sits after the kx=0 block
                    # so it doesn't stall on this band's VT3 copies
                    di = 0
                    for kx in range(3):
                        for ey, ex in INNER:
                            term(nc.vector, ACCA, TMPA, kx, ey, ex, di == 0)
                            di += 1
                        if kx == 0:
                            DUM = pb1.tile([128, 1], BF16, tag="dum")
                            nc.vector.tensor_scalar(
                                out=DUM[:], in0=VT3[:, 0:1], scalar1=1.0,
                                scalar2=None, op0=ALU.mult,
                            )

                    # -- collapse + out (all DVE: keep the scalar queue free of
                    # MAC-dependent ops so next-band copies overlap this MAC)
                    nc.vector.tensor_tensor(
                        out=ACCA[:], in0=ACCA[:], in1=ACCB[:], op=ALU.add
                    )
                    nc.vector.tensor_tensor(
                        out=ACCA[:, 0], in0=ACCA[:, 0], in1=ACCA[:, 1], op=ALU.add
                    )
                    nc.vector.tensor_tensor(
                        out=ACCA[:, 0], in0=ACCA[:, 0], in1=ACCA[:, 2], op=ALU.add
                    )
                    nc.sync.dma_start(
                        outD[:, ib], ACCA[:, 0].rearrange("p a b -> p (a b)")
                    )

    if fix_waits:
        _fix_multiwait(nc)
    return nc


def make_consts(w_off, b_off, w_mod, b_mod, w_reg):
    wconv = np.zeros((128, KK, 27), np.float32)
    for t in range(KK):
        ty, tx = t // 3, t % 3
        wconv[0:64, t, 0:18] = w_off[:, :, ty, tx].T     # z half -> offsets
        wconv[64:128, t, 18:27] = w_mod[:, :, ty, tx].T  # x half -> mask
    # reorder offset channels so planes are [dy*9, dx*9, mask*9]
    perm = list(range(0, 18, 2)) + list(range(1, 18, 2)) + list(range(18, 27))
    wconv = wconv[:, :, perm]
    wcz = wconv[0:64].reshape(64, KK * 27)
    wcx = wconv[64:128].reshape(64, KK * 27)
    w3 = w_reg.reshape(O, C, 3, 3)  # [o, c, ky, kx]
    # x2: the deform-conv mask is 2*sigmoid; fold the 2 into the weights
    wflat = np.ascontiguousarray(
        w3.transpose(1, 3, 2, 0).reshape(C, 3 * 3 * O)) * 2.0
    bias27 = np.concatenate([b_off[perm[:18]], b_mod]).reshape(27, 1).astype(
        np.float32
    )
    identb = np.eye(32, dtype=np.float32)
    cstb = np.tile(
        np.array([-2.0, -1.0, 0.0, 1.0, 2.0, 1.0, -0.99, 0.0], np.float32), (128, 1)
    )
    onesb = np.ones((128, 1), np.float32)
    return wcz, wcx, wflat, bias27, identb, cstb, onesb


def make_pad(img):
    p = np.zeros((64, PW, PW), np.float32)
    p[:, 3 : 3 + H, 3 : 3 + W] = img
    return p.reshape(64, PW * PW)


_NC_CACHE = None


def _get_nc():
    global _NC_CACHE
    if _NC_CACHE is None:
        _NC_CACHE = build_nc()
    return _NC_CACHE


def _make_in_maps(inp):
    import ml_dtypes

    bf = ml_dtypes.bfloat16
    x = np.asarray(inp["x"], np.float32)
    z = np.asarray(inp["z"], np.float32)
    wcz, wcx, wflat, bias27, identb, cstb, onesb = make_consts(
        np.asarray(inp["w_off"], np.float32), np.asarray(inp["b_off"], np.float32),
        np.asarray(inp["w_mod"], np.float32), np.asarray(inp["b_mod"], np.float32),
        np.asarray(inp["w_reg"], np.float32),
    )
    in_maps = []
    for b in range(x.shape[0]):
        in_maps.append(
            dict(
                zin=make_pad(z[b]).astype(bf),
                xin=make_pad(x[b]).astype(bf),
                wcz=wcz.astype(bf),
                wcx=wcx.astype(bf),
                wflat=wflat.astype(bf),
                bias27=bias27,
                identb=identb.astype(bf),
                cstb=cstb.astype(bf),
                onesb=onesb.astype(bf),
            )
        )
    return in_maps


def kernel(x, z, w_off, b_off, w_mod, b_mod, w_reg):
    in_maps = _make_in_maps(
        dict(x=x, z=z, w_off=w_off, b_off=b_off, w_mod=w_mod, b_mod=b_mod,
             w_reg=w_reg)
    )
    nc = _get_nc()
    res = run_bass_kernel_spmd(nc, in_maps, list(range(NCORES)))
    outs = []
    for b in range(len(in_maps)):
        arr = np.asarray(res.results[b]["outD"], np.float32).reshape(128, NBANDS, O, R)
        outs.append(
            np.ascontiguousarray(arr.transpose(2, 1, 3, 0)).reshape(O, H, W)
        )
    return np.stack(outs).astype(np.float32)
